# revision 35
# baseline (speedup 1.0000x reference)
"""Trainium2 Bass kernel for BiquadCellWithSidechain.

Reference recurrence (per time step t, per batch lane b):
    cs[t,b,:] = weights + sidechain[t,b,:]                  (5 taps)
    ff[t,b]   = sum_i x[t,b,i] * cs[t,b,i]   i in 0..2      (feedforward)
    a1[t,b]   = cs[t,b,3] ; a2[t,b] = cs[t,b,4]
    o[t,b]    = tanh(ff[t,b] + a1[t,b]*o[t-1,b] + a2[t,b]*o[t-2,b])

Strategy (v2):
  - Data-parallel over B: 8 cores x 128 lanes (lanes = SBUF partitions).
  - Segmented scan: T=4096 split into S=128 segments of SEG=32 steps.
    Zero-state warmup of L=64 steps reproduces the fp32 trajectory to
    ~7e-6 (fading memory; validated in numpy).  Chain = SEG+L = 96 steps
    of width S, vs 112 in v1.
  - Host marshaling: x/sidechain are supplied as tap-separated, warmup-
    padded, (row=t' mod SEG, col=t' div SEG)-permuted grids (t'=t+L).
    Chain step j then reads coefficient row j%SEG at col offset j//SEG:
    a CONTIGUOUS run of S elements - no strided access, no on-device
    transposes, and phase A is purely elementwise.
  - Phase A streams 8 chunks of 4 grid rows; chain step j only needs
    grid row j%SEG, so the chain starts after the first chunk and
    overlaps the (DMA-bound, ~50us) input streaming.
  - Chain step j: Pool computes u_j = ff_j + a2_j*o_{j-2} two steps
    ahead; DVE computes v = u + a1*o_{j-1} in two ping-ponged halves so
    ACT's tanh of one half overlaps DVE work on the other.
  - o is stored step-major ([lane, block j * S]); output blocks (j>=L)
    are DMA'd out raw and the host de-permutes - zero engine cost.
  - All fp32: the recurrence is locally chaotic on some lanes; any fp16
    rounding anywhere diverges past the 2e-2 gate (validated in numpy).
"""

import numpy as np
from contextlib import ExitStack

import concourse.bass as bass
import concourse.bacc as bacc
import concourse.mybir as mybir
import concourse.tile as tile
from concourse.bass_utils import run_bass_kernel_spmd

F32 = mybir.dt.float32
ALU = mybir.AluOpType
ACTF = mybir.ActivationFunctionType

T = 4096          # time steps
B = 1024          # total batch lanes
NC = 8            # cores
BS = B // NC      # lanes per core = 128 SBUF partitions
NFF = 3
SEG = 32          # segment length
S = T // SEG      # 128 segments (chain op width)
L = 64            # warmup steps (L=64 -> ~7e-6 max err, L=72 bit-exact)
CH = SEG + L      # chain steps = 96
SP_ = S + L // SEG  # grid cols per row = 130
GRID = SEG * SP_  # grid elems per lane = 4160 (= T + L)
RPC = 4           # grid rows per phase-A chunk
NCHUNK = SEG // RPC  # 8 chunks
HB = S // 2       # half width = 64
OBATCH = 4        # output blocks per DMA


DEFAULT_CFG = dict(
    cs="stt",    # (sc_i+w_i)*x_i: 'stt' = fused DVE STT; 'act' = ACT bias-add
                 # + Pool mul; 'pool' = Pool TT pair (wbc tile)
    ff="dve",    # ff = tmp0+tmp1+tmp2: 'dve' | 'pool'
    aa="act",    # a1/a2 = sc+w: 'act' | 'dve' (tensor_scalar)
    u="pool",    # u pair engine: 'pool' | 'dve' (both emitted 2 steps ahead)
)


def build_kernel(reps: int = 1, phases: str = "ABC",
                 cfg: dict | None = None) -> bass.Bass:
    """phases: subset of 'A' (streaming), 'B' (chain + outputs).
    cfg: engine-assignment choices, see DEFAULT_CFG."""
    cfg = {**DEFAULT_CFG, **(cfg or {})}
    nc = bacc.Bacc(num_swdge_queues=4)

    xg_d = nc.declare_dram_parameter("xg", [BS, NFF * GRID], F32, isOutput=False)
    scg_d = nc.declare_dram_parameter("scg", [BS, 5 * GRID], F32, isOutput=False)
    wt_d = nc.declare_dram_parameter("wt", [BS, 8], F32, isOutput=False)
    wbc_d = nc.declare_dram_parameter("wbc", [BS, NFF * RPC * SP_], F32,
                                      isOutput=False)
    c0_d = nc.declare_dram_parameter("c0", [BS, 2], F32, isOutput=False)
    y_d = nc.declare_dram_parameter("y", [BS, SEG * S], F32, isOutput=True)

    xg_v = xg_d.rearrange("p (i r c) -> p i r c", i=NFF, c=SP_)
    scg_v = scg_d.rearrange("p (i r c) -> p i r c", i=5, c=SP_)

    with ExitStack() as ctx:
        tc = ctx.enter_context(tile.TileContext(nc))

        const_pool = ctx.enter_context(tc.tile_pool(name="const", bufs=1))
        grid_pool = ctx.enter_context(tc.tile_pool(name="grid", bufs=1))
        in_pool = ctx.enter_context(tc.tile_pool(name="inp", bufs=2))
        work_pool = ctx.enter_context(tc.tile_pool(name="work", bufs=2))
        chain_pool = ctx.enter_context(tc.tile_pool(name="chain", bufs=6))

        wt = const_pool.tile([BS, 8], F32)
        nc.sync.dma_start(wt[:], wt_d[:, :])
        wbc = const_pool.tile([BS, NFF * RPC * SP_], F32)
        nc.sync.dma_start(wbc[:], wbc_d[:, :])
        wbc_v = wbc[:].rearrange("p (i r c) -> p i r c", i=NFF, c=SP_)

        # coefficient grids, [lane, row, col]
        ff_g = grid_pool.tile([BS, GRID], F32)
        a1_g = grid_pool.tile([BS, GRID], F32)
        a2_g = grid_pool.tile([BS, GRID], F32)
        ff_v = ff_g[:].rearrange("p (r c) -> p r c", c=SP_)
        a1_v = a1_g[:].rearrange("p (r c) -> p r c", c=SP_)
        a2_v = a2_g[:].rearrange("p (r c) -> p r c", c=SP_)

        # o, step-major: chain block j lives at cols (j+2)*S; blocks at
        # j=-2,-1 are the zero warmup carries.
        o_g = grid_pool.tile([BS, (CH + 2) * S], F32)

        def o_blk(j):
            return o_g[:, (j + 2) * S : (j + 3) * S]

        # preload tanh table early
        warm = const_pool.tile([128, 1], F32)
        nc.scalar.memzero(warm[:])
        nc.scalar.activation(warm[:], warm[:], ACTF.Tanh)

        def crow(view, j):
            # contiguous S-wide coefficient slice for chain step j
            return view[:, j % SEG, j // SEG : j // SEG + S]

        for _rep in range(reps):
            nc.vector.memset(o_g[:, 0 : 2 * S], 0.0)
            if "A" not in phases:
                # chain-only benchmarking: init grids so reads are defined
                nc.vector.memset(ff_g[:], 0.0)
                nc.vector.memset(a1_g[:], 0.0)
                nc.vector.memset(a2_g[:], 0.0)

            # ---------------- Phase A: one chunk of 4 grid rows ----------
            def emit_chunk(k):
                r0 = k * RPC
                xq = in_pool.tile([BS, NFF * RPC * SP_], F32, tag="xq")
                xqv = xq[:].rearrange("p (i r c) -> p i r c", i=NFF, c=SP_)
                nc.sync.dma_start(xqv, xg_v[:, :, r0 : r0 + RPC, :])
                scq = in_pool.tile([BS, 5 * RPC * SP_], F32, tag="scq")
                scqv = scq[:].rearrange("p (i r c) -> p i r c", i=5, c=SP_)
                nc.sync.dma_start(scqv, scg_v[:, :, r0 : r0 + RPC, :])

                # tmp_i = (sc_i + w_i) * x_i
                tmp = work_pool.tile([BS, NFF * RPC * SP_], F32, tag="tmp")
                tmpv = tmp[:].rearrange("p (i r c) -> p i r c", i=NFF, c=SP_)
                if cfg["cs"] == "stt":
                    for i in range(NFF):
                        nc.vector.scalar_tensor_tensor(
                            tmpv[:, i], scqv[:, i], wt[:, i : i + 1],
                            xqv[:, i], ALU.add, ALU.mult)
                elif cfg["cs"] == "act":
                    for i in range(NFF):
                        nc.scalar.activation(tmpv[:, i], scqv[:, i],
                                             ACTF.Identity,
                                             bias=wt[:, i : i + 1])
                    for r in range(RPC):
                        nc.gpsimd.tensor_mul(tmpv[:, :, r], tmpv[:, :, r],
                                             xqv[:, :, r])
                else:  # pool
                    for r in range(RPC):
                        nc.gpsimd.tensor_add(tmpv[:, :, r], scqv[:, 0:NFF, r],
                                             wbc_v[:, :, r])
                        nc.gpsimd.tensor_mul(tmpv[:, :, r], tmpv[:, :, r],
                                             xqv[:, :, r])
                # ff rows = tmp0 + tmp1 + tmp2
                ffs = ff_v[:, r0 : r0 + RPC, :]
                eng_ff = nc.vector if cfg["ff"] == "dve" else nc.gpsimd
                eng_ff.tensor_add(ffs, tmpv[:, 0], tmpv[:, 1])
                eng_ff.tensor_add(ffs, ffs, tmpv[:, 2])
                # a1/a2 rows = sc_3/4 + w_3/4
                if cfg["aa"] == "act":
                    nc.scalar.activation(a1_v[:, r0 : r0 + RPC, :],
                                         scqv[:, 3], ACTF.Identity,
                                         bias=wt[:, 3:4])
                    nc.scalar.activation(a2_v[:, r0 : r0 + RPC, :],
                                         scqv[:, 4], ACTF.Identity,
                                         bias=wt[:, 4:5])
                else:
                    nc.vector.tensor_scalar(a1_v[:, r0 : r0 + RPC, :],
                                            scqv[:, 3], wt[:, 3:4], None,
                                            ALU.add)
                    nc.vector.tensor_scalar(a2_v[:, r0 : r0 + RPC, :],
                                            scqv[:, 4], wt[:, 4:5], None,
                                            ALU.add)

            # ---------------- Phase B: one chain step --------------------
            # step j, segment s -> t = s*SEG + j - L; coeff slice crow(j);
            # o_{t-1} = block j-1, o_{t-2} = block j-2.
            # u_j = ff_j + a2_j * o_{j-2}, on DVE, emitted at the TAIL of
            # iteration j-1 (deps are then long satisfied, so the pair
            # never blocks the critical v-ops in DVE's in-order stream and
            # u never takes a cross-engine sem round-trip).  Pool is left
            # entirely free for phase A.
            u_tiles = {}

            def emit_u(j):
                eng = nc.vector if cfg["u"] == "dve" else nc.gpsimd
                u = chain_pool.tile([BS, S], F32, tag=f"u{j % 3}")
                eng.tensor_mul(u[:], crow(a2_v, j), o_blk(j - 2))
                eng.tensor_add(u[:], u[:], crow(ff_v, j))
                u_tiles[j] = u

            def emit_step(j):
                u = u_tiles.pop(j)
                for h in (0, 1):
                    sl = slice(h * HB, (h + 1) * HB)
                    v = chain_pool.tile([BS, HB], F32, tag=f"v{h}")
                    nc.vector.tensor_mul(v[:], crow(a1_v, j)[:, sl],
                                         o_blk(j - 1)[:, sl])
                    nc.vector.tensor_add(v[:], v[:], u[:, sl])
                    nc.scalar.activation(o_blk(j)[:, sl], v[:], ACTF.Tanh)

                # true carry seed for segment 0 (t=-1 -> block L-1, s=0;
                # t=-2 -> block L-2, s=0), overwriting the (zero) warmup
                # value right after it is produced.
                if j == L - 2:
                    nc.sync.dma_start(o_blk(L - 2)[:, 0:1], c0_d[:, 1:2])
                if j == L - 1:
                    nc.sync.dma_start(o_blk(L - 1)[:, 0:1], c0_d[:, 0:1])

                # ---------------- Phase C: stream outputs ----------------
                if j >= L and (j - L) % OBATCH == OBATCH - 1:
                    rr = j - L - (OBATCH - 1)
                    nc.sync.dma_start(
                        y_d[:, rr * S : (rr + OBATCH) * S],
                        o_g[:, (L + 2 + rr) * S : (L + 2 + rr + OBATCH) * S])

            # Interleaved emission: chain step j only needs grid rows
            # j % SEG (chunk (j % SEG) // RPC), so steps 4k..4k+3 are
            # emitted right after chunk k -- the in-order engine streams
            # then overlap the chain prefix with the (DMA-bound) load.
            # u(j) may only be emitted once BOTH its inputs exist in program
            # order: grid row j%SEG (chunk (j%SEG)//RPC) and o block j-2
            # (step j-2).  Emitting it earlier is a read-before-write race
            # (Tile cannot depend on future writes).
            emitted_u: set = set()

            def ensure_u(j):
                if j < CH and j not in emitted_u:
                    emit_u(j)
                    emitted_u.add(j)

            if "B" in phases and "A" in phases:
                for k in range(NCHUNK):
                    emit_chunk(k)
                    ensure_u(k * RPC)
                    ensure_u(k * RPC + 1)
                    for j in range(k * RPC, (k + 1) * RPC):
                        emit_step(j)
                        if (j + 2) % SEG < (k + 1) * RPC:
                            ensure_u(j + 2)
                ensure_u(NCHUNK * RPC)
                ensure_u(NCHUNK * RPC + 1)
                for j in range(NCHUNK * RPC, CH):
                    emit_step(j)
                    ensure_u(j + 2)
            elif "A" in phases:
                for k in range(NCHUNK):
                    emit_chunk(k)
            elif "B" in phases:
                ensure_u(0)
                ensure_u(1)
                for j in range(CH):
                    emit_step(j)
                    ensure_u(j + 2)

    return nc


_CACHE: dict = {}


def _get_nc() -> bass.Bass:
    if "nc" not in _CACHE:
        nc = build_kernel()
        if not nc.is_finalized():
            nc.finalize()
        _CACHE["nc"] = nc
    return _CACHE["nc"]


def _permute_grid(a):
    """[T, BS, taps] -> [BS, taps, SEG, SP_] warmup-padded permuted grid.

    grid[lane, i, rho, c] = a[t, lane, i] with t = c*SEG + rho - L (zeros
    for t < 0).
    """
    Tn, BSn, taps = a.shape
    pad = np.zeros((L, BSn, taps), a.dtype)
    ap = np.concatenate([pad, a], axis=0)            # [T+L, BS, taps]
    g = ap.reshape(SP_, SEG, BSn, taps)              # [c, rho, lane, i]
    return np.ascontiguousarray(g.transpose(2, 3, 1, 0))  # [lane, i, rho, c]


def make_in_maps(x, sidechain, carry0, weights):
    x = np.asarray(x, np.float32)
    sidechain = np.asarray(sidechain, np.float32)
    carry0 = np.asarray(carry0, np.float32)
    weights = np.asarray(weights, np.float32)
    w_flat = weights.reshape(5)
    wt = np.zeros((BS, 8), np.float32)
    wt[:, 0:5] = w_flat
    wbc = np.ascontiguousarray(np.broadcast_to(
        w_flat[0:NFF, None, None], (NFF, RPC, SP_))[None].repeat(BS, 0)
    ).reshape(BS, NFF * RPC * SP_)
    in_maps = []
    for c in range(NC):
        lo, hi = c * BS, (c + 1) * BS
        in_maps.append({
            "xg": _permute_grid(x[:, lo:hi, :]).reshape(BS, NFF * GRID),
            "scg": _permute_grid(sidechain[:, lo:hi, :]).reshape(BS, 5 * GRID),
            "wt": wt,
            "wbc": wbc,
            # col 0 = o_{t=-1} = carry0[:,0]; col 1 = o_{t=-2} = carry0[:,1]
            "c0": np.ascontiguousarray(carry0[lo:hi, :]),
        })
    return in_maps


def kernel(x: np.ndarray, sidechain: np.ndarray, carry0: np.ndarray,
           weights: np.ndarray) -> np.ndarray:
    nc = _get_nc()
    in_maps = make_in_maps(x, sidechain, carry0, weights)
    res = run_bass_kernel_spmd(nc, in_maps, list(range(NC)))
    out = np.empty((T, B, 1), np.float32)
    for c in range(NC):
        # y[lane, r*S + s] = o at t = s*SEG + r; de-permute to [t, lane]
        yc = res.results[c]["y"].reshape(BS, SEG, S)     # [lane, r, s]
        out[:, c * BS : (c + 1) * BS, 0] = (
            yc.transpose(2, 1, 0).reshape(T, BS))        # [s, r, lane]
    return out
